# revision 10
# baseline (speedup 1.0000x reference)
"""Causal multi-head attention on 8 Trainium2 NeuronCores.

Problem: B=4, H=16, S=2048, D=128, f32, causal mask.
Sharding: batch*heads (64 pairs) split across 8 cores, 8 pairs each.

Per-core algorithm ("transposed flash", no on-device transposes):
  - Host pre-transposes Q,K to D-major [D, S] (f32, fed to PE as float32r)
    and converts V to bf16 [S, D].
  - S^T[k, q] blocks ([128k x 512q]) = matmul(lhsT=K^T chunk, rhs=Q^T chunk),
    fp32r at N=512 -> 1 cycle/row.
  - exp on ScalarE with the 1/sqrt(D) scale folded in free; output bf16.
  - Mask applied multiplicatively AFTER exp, only on blocks that the host
    classified as partially masked (for the causal mask: the 16 diagonal
    blocks). Fully-masked blocks are skipped entirely at compile time.
  - out^T[d, q] += matmul(lhsT=V block (natural layout!), rhs=P^T block).
  - denominator[q] += matmul(lhsT=ones[128,1], rhs=P^T block) in PSUM.
  - Host divides out^T by den and transposes back to [S, D].
"""

import math
import numpy as np
import ml_dtypes

B, H, S, D = 4, 16, 2048, 128
N_CORES = 8
BH = B * H
PAIRS = BH // N_CORES          # (b,h) pairs per core
QCHUNK = 512                   # q columns per PSUM accumulation chunk
NQC = S // QCHUNK              # 4
KBLK = 128                     # k rows per block (PE contraction/partition)
NKB = S // KBLK                # 16

_BF16 = ml_dtypes.bfloat16


def _classify_blocks(mask):
    """Per (qc, kb) block: 'full' (no mask), 'partial' (band tile), or skip.

    mask[q, k] True = masked out. Shared across batch/heads, so the block
    structure is compile-time for the whole kernel.
    Returns (blocks_by_qc, band_tiles) where blocks_by_qc[qc] is a list of
    (kb, band_idx_or_None) and band_tiles is [T, 128, 512] bf16 {0,1}
    keep-mask tiles transposed to [k, q].
    """
    keep = ~np.asarray(mask)
    band_tiles = []
    blocks_by_qc = []
    for qc in range(NQC):
        blocks = []
        qs = slice(qc * QCHUNK, (qc + 1) * QCHUNK)
        for kb in range(NKB):
            ks = slice(kb * KBLK, (kb + 1) * KBLK)
            blk = keep[qs, ks]  # [512 q, 128 k]
            if not blk.any():
                continue
            # narrowest q-window [off, 512) containing all unmasked entries;
            # fp32r matmuls need N >= 256 to run at 1 cycle/row, so floor w.
            cols = blk.any(axis=1)
            off = int(np.argmax(cols))
            off = min(off, QCHUNK - 256)
            off -= off % 128
            w = QCHUNK - off
            sub = blk[off:, :]
            if sub.all():
                blocks.append((kb, off, w, None))
            else:
                band_tiles.append(np.ascontiguousarray(blk.T).astype(_BF16))
                blocks.append((kb, off, w, len(band_tiles) - 1))
        blocks_by_qc.append(blocks)
    if not band_tiles:
        band_tiles = [np.zeros((KBLK, QCHUNK), dtype=_BF16)]
    return blocks_by_qc, np.stack(band_tiles)


def _split_big_waits(nc, mybir, max_waits=1):
    """Walrus in this container accepts only one sync-wait command per
    instruction; split extras onto preceding NoOps on the same engine."""
    for f in nc.m.functions:
        for blk in f.blocks:
            new_insts = []
            for inst in blk.instructions:
                si = inst.sync_info
                if si is not None and si.on_wait and len(si.on_wait) > max_waits:
                    waits = list(si.on_wait)
                    extra, keep = waits[:-max_waits], waits[-max_waits:]
                    for i in range(0, len(extra), max_waits):
                        nop = mybir.InstNoOp(
                            name=nc.get_next_instruction_name(),
                            engine=inst.engine,
                            ins=[], outs=[],
                            sync_info=mybir.SyncInfo(
                                on_wait=extra[i:i + max_waits], on_update=[]),
                        )
                        new_insts.append(nop)
                    inst.sync_info = mybir.SyncInfo(
                        on_wait=keep, on_update=list(si.on_update or []))
                new_insts.append(inst)
            blk.instructions[:] = new_insts


def _build(blocks_by_qc, n_band):
    import concourse.bass as bass
    import concourse.mybir as mybir
    import concourse.tile as tile

    nc = bass.Bass()
    qT_d = nc.declare_dram_parameter("qT", [PAIRS, D, S], mybir.dt.float32r, isOutput=False)
    kT_d = nc.declare_dram_parameter("kT", [PAIRS, D, S], mybir.dt.float32r, isOutput=False)
    v_d = nc.declare_dram_parameter("v", [PAIRS, S, D], mybir.dt.bfloat16, isOutput=False)
    band_d = nc.declare_dram_parameter("band", [n_band, KBLK, QCHUNK], mybir.dt.bfloat16, isOutput=False)
    outT_d = nc.declare_dram_parameter("outT", [PAIRS, D, S], mybir.dt.float32, isOutput=True)
    den_d = nc.declare_dram_parameter("den", [PAIRS, S], mybir.dt.float32, isOutput=True)

    inv_sqrt_d = 1.0 / math.sqrt(D)

    with tile.TileContext(nc) as tc:
        with (
            tc.tile_pool(name="qk", bufs=2) as qk_pool,
            tc.tile_pool(name="vp", bufs=2) as v_pool,
            tc.tile_pool(name="pt", bufs=6) as pt_pool,
            tc.tile_pool(name="aux", bufs=1) as aux_pool,
            tc.tile_pool(name="osb", bufs=3) as osb_pool,
            tc.tile_pool(name="st_ps", bufs=3, space="PSUM") as st_psum,
            tc.tile_pool(name="o_ps", bufs=1, space="PSUM") as o_psum,
            tc.tile_pool(name="d_ps", bufs=1, space="PSUM") as d_psum,
        ):
            band_sb = aux_pool.tile([KBLK, n_band, QCHUNK], mybir.dt.bfloat16, tag="band")
            band_src = band_d.rearrange("t p q -> p t q")
            for c in range(0, n_band, 4):
                ce = min(c + 4, n_band)
                nc.gpsimd.dma_start(out=band_sb[:, c:ce, :], in_=band_src[:, c:ce, :])
            ones_sb = aux_pool.tile([128, 128], mybir.dt.bfloat16, tag="ones")
            nc.vector.memset(ones_sb[:], 1.0)

            for i in range(PAIRS):
                qT = qk_pool.tile([D, S], mybir.dt.float32r, tag="qT")
                kT = qk_pool.tile([D, S], mybir.dt.float32r, tag="kT")
                # chunked loads so the first QK matmuls can start after
                # the first quarter of each tensor lands
                for c in range(NQC):
                    nc.sync.dma_start(out=qT[:, bass.ts(c, QCHUNK)], in_=qT_d[i][:, bass.ts(c, QCHUNK)])
                    nc.sync.dma_start(out=kT[:, bass.ts(c, QCHUNK)], in_=kT_d[i][:, bass.ts(c, QCHUNK)])
                # v natural [S, D] -> [128 k-part, kb, D]
                vt = v_pool.tile([KBLK, NKB, D], mybir.dt.bfloat16, tag="v")
                nc.sync.dma_start(
                    out=vt[:], in_=v_d[i].rearrange("(kb p) d -> p kb d", p=KBLK)
                )

                den_sb = osb_pool.tile([1, S], mybir.dt.float32, tag="densb")
                for qc in range(NQC):
                    blocks = blocks_by_qc[qc]
                    qsl = bass.ts(qc, QCHUNK)
                    o_acc = o_psum.tile([D, QCHUNK], mybir.dt.float32, tag="oacc")
                    # M=128 (every row = denominator): narrow-M matmuls
                    # (M<128) break PE pipelining, costing ~+100ns on every
                    # matmul around them; full-M runs at streaming rate.
                    den_acc = d_psum.tile([D, QCHUNK], mybir.dt.float32, tag="dacc")
                    nblk = len(blocks)
                    # greedy-pack blocks into psum units of <= 1024 columns
                    units = []
                    cur, cw = [], 0
                    for blk in blocks:
                        if cur and cw + blk[2] > 2 * QCHUNK:
                            units.append(cur)
                            cur, cw = [], 0
                        cur.append((blk, cw))
                        cw += blk[2]
                    if cur:
                        units.append(cur)
                    bi = 0  # global block index within qc
                    for unit in units:
                        uw = sum(b[0][2] for b in unit)
                        sT = st_psum.tile([KBLK, 2 * QCHUNK], mybir.dt.float32, tag="sT")
                        for (kb, off, w, _), po in unit:
                            nc.tensor.matmul(
                                sT[:, po:po + w],
                                lhsT=kT[:, bass.ts(kb, KBLK)],
                                rhs=qT[:, qc * QCHUNK + off: qc * QCHUNK + off + w],
                                start=True, stop=True,
                            )
                        pT = pt_pool.tile([KBLK, 2 * QCHUNK], mybir.dt.bfloat16, tag="pT")
                        nc.scalar.activation(
                            pT[:, 0:uw], sT[:, 0:uw],
                            mybir.ActivationFunctionType.Exp,
                            scale=inv_sqrt_d,
                        )
                        for (kb, off, w, band_idx), po in unit:
                            if band_idx is not None:
                                nc.vector.tensor_mul(
                                    pT[:, po:po + w],
                                    pT[:, po:po + w],
                                    band_sb[:, band_idx, off:off + w],
                                )
                        for (kb, off, w, _), po in unit:
                            first = (bi == 0)
                            last = (bi == nblk - 1)
                            bi += 1
                            nc.tensor.matmul(
                                o_acc[:, off:off + w],
                                lhsT=vt[:, kb, :],
                                rhs=pT[:, po:po + w],
                                start=first, stop=last,
                            )
                            nc.tensor.matmul(
                                den_acc[:, off:off + w],
                                lhsT=ones_sb[:],
                                rhs=pT[:, po:po + w],
                                start=first, stop=last,
                            )
                    o_sb = osb_pool.tile([D, QCHUNK], mybir.dt.float32, tag="osb")
                    nc.vector.tensor_copy(o_sb[:], o_acc[:])
                    nc.sync.dma_start(out=outT_d[i][:, qsl], in_=o_sb[:])
                    nc.vector.tensor_copy(den_sb[:, qsl], den_acc[0:1, :])
                nc.sync.dma_start(out=den_d[i:i + 1, :], in_=den_sb[:])

    _split_big_waits(nc, mybir)
    return nc


def kernel(k, q, v, mask):
    from concourse.bass_utils import run_bass_kernel_spmd

    k = np.asarray(k, dtype=np.float32).reshape(BH, S, D)
    q = np.asarray(q, dtype=np.float32).reshape(BH, S, D)
    v = np.asarray(v, dtype=np.float32).reshape(BH, S, D)

    qT = np.ascontiguousarray(q.transpose(0, 2, 1))            # [BH, D, S] f32
    kT = np.ascontiguousarray(k.transpose(0, 2, 1))            # [BH, D, S] f32
    vb = v.astype(_BF16)                                       # [BH, S, D] bf16

    blocks_by_qc, band = _classify_blocks(mask)
    nc = _build(blocks_by_qc, band.shape[0])

    in_maps = []
    for c in range(N_CORES):
        sl = slice(c * PAIRS, (c + 1) * PAIRS)
        in_maps.append({
            "qT": qT[sl], "kT": kT[sl], "v": vb[sl], "band": band,
        })
    res = run_bass_kernel_spmd(nc, in_maps, core_ids=list(range(N_CORES)))

    outT = np.stack([res.results[c]["outT"] for c in range(N_CORES)])  # [C, PAIRS, D, S]
    den = np.stack([res.results[c]["den"] for c in range(N_CORES)])    # [C, PAIRS, S]
    out = outT.reshape(BH, D, S).transpose(0, 2, 1) / den.reshape(BH, S)[:, :, None]
    return out.reshape(B, H, S, D).astype(np.float32)


# revision 11
# speedup vs baseline: 1.0921x; 1.0921x over previous
"""Causal multi-head attention on 8 Trainium2 NeuronCores.

Problem: B=4, H=16, S=2048, D=128, f32, causal mask.
Sharding: batch*heads (64 pairs) split across 8 cores, 8 pairs each.

Per-core algorithm ("transposed flash", no on-device transposes):
  - Host pre-transposes Q,K to D-major [D, S] (f32, fed to PE as float32r)
    and converts V to bf16 [S, D].
  - S^T[k, q] blocks ([128k x 512q]) = matmul(lhsT=K^T chunk, rhs=Q^T chunk),
    fp32r at N=512 -> 1 cycle/row.
  - exp on ScalarE with the 1/sqrt(D) scale folded in free; output bf16.
  - Mask applied multiplicatively AFTER exp, only on blocks that the host
    classified as partially masked (for the causal mask: the 16 diagonal
    blocks). Fully-masked blocks are skipped entirely at compile time.
  - out^T[d, q] += matmul(lhsT=V block (natural layout!), rhs=P^T block).
  - denominator[q] += matmul(lhsT=ones[128,1], rhs=P^T block) in PSUM.
  - Host divides out^T by den and transposes back to [S, D].
"""

import math
import numpy as np
import ml_dtypes

B, H, S, D = 4, 16, 2048, 128
N_CORES = 8
BH = B * H
PAIRS = BH // N_CORES          # (b,h) pairs per core
QCHUNK = 512                   # q columns per PSUM accumulation chunk
NQC = S // QCHUNK              # 4
KBLK = 128                     # k rows per block (PE contraction/partition)
NKB = S // KBLK                # 16

_BF16 = ml_dtypes.bfloat16


def _classify_blocks(mask):
    """Per (qc, kb) block: 'full' (no mask), 'partial' (band tile), or skip.

    mask[q, k] True = masked out. Shared across batch/heads, so the block
    structure is compile-time for the whole kernel.
    Returns (blocks_by_qc, band_tiles) where blocks_by_qc[qc] is a list of
    (kb, band_idx_or_None) and band_tiles is [T, 128, 512] bf16 {0,1}
    keep-mask tiles transposed to [k, q].
    """
    keep = ~np.asarray(mask)
    band_tiles = []
    blocks_by_qc = []
    for qc in range(NQC):
        blocks = []
        qs = slice(qc * QCHUNK, (qc + 1) * QCHUNK)
        for kb in range(NKB):
            ks = slice(kb * KBLK, (kb + 1) * KBLK)
            blk = keep[qs, ks]  # [512 q, 128 k]
            if not blk.any():
                continue
            if blk.all():
                blocks.append((kb, 0, QCHUNK, None))
            else:
                band_tiles.append(np.ascontiguousarray(blk.T).astype(_BF16))
                blocks.append((kb, 0, QCHUNK, len(band_tiles) - 1))
        blocks_by_qc.append(blocks)
    if not band_tiles:
        band_tiles = [np.zeros((KBLK, QCHUNK), dtype=_BF16)]
    return blocks_by_qc, np.stack(band_tiles)


def _split_big_waits(nc, mybir, max_waits=1):
    """Walrus in this container accepts only one sync-wait command per
    instruction; split extras onto preceding NoOps on the same engine."""
    for f in nc.m.functions:
        for blk in f.blocks:
            new_insts = []
            for inst in blk.instructions:
                si = inst.sync_info
                if si is not None and si.on_wait and len(si.on_wait) > max_waits:
                    waits = list(si.on_wait)
                    extra, keep = waits[:-max_waits], waits[-max_waits:]
                    for i in range(0, len(extra), max_waits):
                        nop = mybir.InstNoOp(
                            name=nc.get_next_instruction_name(),
                            engine=inst.engine,
                            ins=[], outs=[],
                            sync_info=mybir.SyncInfo(
                                on_wait=extra[i:i + max_waits], on_update=[]),
                        )
                        new_insts.append(nop)
                    inst.sync_info = mybir.SyncInfo(
                        on_wait=keep, on_update=list(si.on_update or []))
                new_insts.append(inst)
            blk.instructions[:] = new_insts


def _build(blocks_by_qc, n_band):
    import concourse.bass as bass
    import concourse.mybir as mybir
    import concourse.tile as tile

    nc = bass.Bass()
    qT_d = nc.declare_dram_parameter("qT", [PAIRS, D, S], mybir.dt.float32r, isOutput=False)
    kT_d = nc.declare_dram_parameter("kT", [PAIRS, D, S], mybir.dt.float32r, isOutput=False)
    v_d = nc.declare_dram_parameter("v", [PAIRS, S, D], mybir.dt.bfloat16, isOutput=False)
    band_d = nc.declare_dram_parameter("band", [n_band, KBLK, QCHUNK], mybir.dt.bfloat16, isOutput=False)
    outT_d = nc.declare_dram_parameter("outT", [PAIRS, D, S], mybir.dt.float32, isOutput=True)
    den_d = nc.declare_dram_parameter("den", [PAIRS, S], mybir.dt.float32, isOutput=True)

    inv_sqrt_d = 1.0 / math.sqrt(D)

    with tile.TileContext(nc) as tc:
        with (
            tc.tile_pool(name="qk", bufs=2) as qk_pool,
            tc.tile_pool(name="vp", bufs=2) as v_pool,
            tc.tile_pool(name="pt", bufs=6) as pt_pool,
            tc.tile_pool(name="aux", bufs=1) as aux_pool,
            tc.tile_pool(name="osb", bufs=3) as osb_pool,
            tc.tile_pool(name="st_ps", bufs=3, space="PSUM") as st_psum,
            tc.tile_pool(name="o_ps", bufs=1, space="PSUM") as o_psum,
            tc.tile_pool(name="d_ps", bufs=1, space="PSUM") as d_psum,
        ):
            band_sb = aux_pool.tile([KBLK, n_band, QCHUNK], mybir.dt.bfloat16, tag="band")
            band_src = band_d.rearrange("t p q -> p t q")
            for c in range(0, n_band, 4):
                ce = min(c + 4, n_band)
                nc.gpsimd.dma_start(out=band_sb[:, c:ce, :], in_=band_src[:, c:ce, :])
            ones_sb = aux_pool.tile([128, 128], mybir.dt.bfloat16, tag="ones")
            nc.vector.memset(ones_sb[:], 1.0)

            for i in range(PAIRS):
                qT = qk_pool.tile([D, S], mybir.dt.float32r, tag="qT")
                kT = qk_pool.tile([D, S], mybir.dt.float32r, tag="kT")
                # chunked loads so the first QK matmuls can start after
                # the first quarter of each tensor lands
                for c in range(NQC):
                    nc.sync.dma_start(out=qT[:, bass.ts(c, QCHUNK)], in_=qT_d[i][:, bass.ts(c, QCHUNK)])
                    nc.sync.dma_start(out=kT[:, bass.ts(c, QCHUNK)], in_=kT_d[i][:, bass.ts(c, QCHUNK)])
                # v natural [S, D] -> [128 k-part, kb, D]
                vt = v_pool.tile([KBLK, NKB, D], mybir.dt.bfloat16, tag="v")
                nc.sync.dma_start(
                    out=vt[:], in_=v_d[i].rearrange("(kb p) d -> p kb d", p=KBLK)
                )

                den_sb = osb_pool.tile([1, S], mybir.dt.float32, tag="densb")
                for qc in range(NQC):
                    blocks = blocks_by_qc[qc]
                    qsl = bass.ts(qc, QCHUNK)
                    o_acc = o_psum.tile([D, QCHUNK], mybir.dt.float32, tag="oacc")
                    # M=128 (every row = denominator): narrow-M matmuls
                    # (M<128) break PE pipelining, costing ~+100ns on every
                    # matmul around them; full-M runs at streaming rate.
                    den_acc = d_psum.tile([D, QCHUNK], mybir.dt.float32, tag="dacc")
                    nblk = len(blocks)
                    # greedy-pack blocks into psum units of <= 1024 columns
                    units = []
                    cur, cw = [], 0
                    for blk in blocks:
                        if cur and cw + blk[2] > 2 * QCHUNK:
                            units.append(cur)
                            cur, cw = [], 0
                        cur.append((blk, cw))
                        cw += blk[2]
                    if cur:
                        units.append(cur)
                    bi = 0  # global block index within qc
                    for unit in units:
                        uw = sum(b[0][2] for b in unit)
                        sT = st_psum.tile([KBLK, 2 * QCHUNK], mybir.dt.float32, tag="sT")
                        for (kb, off, w, _), po in unit:
                            nc.tensor.matmul(
                                sT[:, po:po + w],
                                lhsT=kT[:, bass.ts(kb, KBLK)],
                                rhs=qT[:, qc * QCHUNK + off: qc * QCHUNK + off + w],
                                start=True, stop=True,
                            )
                        pT = pt_pool.tile([KBLK, 2 * QCHUNK], mybir.dt.bfloat16, tag="pT")
                        nc.scalar.activation(
                            pT[:, 0:uw], sT[:, 0:uw],
                            mybir.ActivationFunctionType.Exp,
                            scale=inv_sqrt_d,
                        )
                        for (kb, off, w, band_idx), po in unit:
                            if band_idx is not None:
                                nc.vector.tensor_mul(
                                    pT[:, po:po + w],
                                    pT[:, po:po + w],
                                    band_sb[:, band_idx, off:off + w],
                                )
                        for (kb, off, w, _), po in unit:
                            first = (bi == 0)
                            last = (bi == nblk - 1)
                            bi += 1
                            nc.tensor.matmul(
                                o_acc[:, off:off + w],
                                lhsT=vt[:, kb, :],
                                rhs=pT[:, po:po + w],
                                start=first, stop=last,
                            )
                            nc.tensor.matmul(
                                den_acc[:, off:off + w],
                                lhsT=ones_sb[:],
                                rhs=pT[:, po:po + w],
                                start=first, stop=last,
                            )
                    o_sb = osb_pool.tile([D, QCHUNK], mybir.dt.float32, tag="osb")
                    nc.vector.tensor_copy(o_sb[:], o_acc[:])
                    nc.sync.dma_start(out=outT_d[i][:, qsl], in_=o_sb[:])
                    nc.vector.tensor_copy(den_sb[:, qsl], den_acc[0:1, :])
                nc.sync.dma_start(out=den_d[i:i + 1, :], in_=den_sb[:])

    _split_big_waits(nc, mybir)
    return nc


def kernel(k, q, v, mask):
    from concourse.bass_utils import run_bass_kernel_spmd

    k = np.asarray(k, dtype=np.float32).reshape(BH, S, D)
    q = np.asarray(q, dtype=np.float32).reshape(BH, S, D)
    v = np.asarray(v, dtype=np.float32).reshape(BH, S, D)

    qT = np.ascontiguousarray(q.transpose(0, 2, 1))            # [BH, D, S] f32
    kT = np.ascontiguousarray(k.transpose(0, 2, 1))            # [BH, D, S] f32
    vb = v.astype(_BF16)                                       # [BH, S, D] bf16

    blocks_by_qc, band = _classify_blocks(mask)
    nc = _build(blocks_by_qc, band.shape[0])

    in_maps = []
    for c in range(N_CORES):
        sl = slice(c * PAIRS, (c + 1) * PAIRS)
        in_maps.append({
            "qT": qT[sl], "kT": kT[sl], "v": vb[sl], "band": band,
        })
    res = run_bass_kernel_spmd(nc, in_maps, core_ids=list(range(N_CORES)))

    outT = np.stack([res.results[c]["outT"] for c in range(N_CORES)])  # [C, PAIRS, D, S]
    den = np.stack([res.results[c]["den"] for c in range(N_CORES)])    # [C, PAIRS, S]
    out = outT.reshape(BH, D, S).transpose(0, 2, 1) / den.reshape(BH, S)[:, :, None]
    return out.reshape(B, H, S, D).astype(np.float32)
